# revision 25
# baseline (speedup 1.0000x reference)
"""AngularMarginLoss (ArcFace-style) distributed Trainium2 kernel.

Strategy (class-parallel over 8 NeuronCores):
  - Host: L2-normalize x rows and W rows (f64), cast to bf16, transpose to
    [E, *] layouts, shard W columns (classes) across 8 cores, zero-pad each
    shard 12500 -> 12800 columns (25 x 512, PSUM-bank aligned; pad cols give
    exp(0)=1.0 exactly, subtracted on host).
  - Device (per core):
      bf16 matmul (xnT tile stationary, WnT chunk moving) -> PSUM f32
      [128, 2048] groups (4 banks, double buffered)
      ScalarE: exp(30*cos) PSUM->SBUF bf16 + accum_out -> per-group sums
      VectorE: tensor_tensor max tree (bf16 2x mode) + MAX8 -> per-group max
  - Host epilogue: combine partial sums (pad-corrected); resolve argmax by
    rescoring candidate groups (within exp(-30*3e-3) of max) in f64 — this
    absorbs all bf16/exp rounding; compute margin numerator and loss.

Self-contained: shapes hardcoded; device work runs in a subprocess so the
harness's jax state/env cannot interfere.
"""
import os
import subprocess
import sys
import tempfile

import numpy as np

B, E, C = 1024, 256, 100000
NCORES = 8
CS = C // NCORES            # 12500 real classes per core
CSP = 12800                 # padded (25 x 512)
NCCH = 512                  # classes per matmul (ISA cap: 512 out elements)
GFULL = 2048                # full group = 4 matmuls = 4 banks
NGRP = 7                    # per tile: 6 x 2048 + 1 x 512
GROUP_RANGES = [(g * GFULL, GFULL) for g in range(6)] + [(6 * GFULL, 512)]
NTILES = B // 128           # 8 B-tiles
NG_TOT = NTILES * NGRP      # 56
NEB = 6                     # exp buffers
SCALE = 30.0
MARGIN = 0.5
EPS = 1e-6
DELTA_COS = 3e-3            # bf16 error margin for argmax group candidates


def _build_graph():
    import concourse.bass as bass
    import concourse.mybir as mybir

    nc = bass.Bass(target_bir_lowering=False)
    bf = mybir.dt.bfloat16
    f32 = mybir.dt.float32
    xt_ext = nc.declare_dram_parameter("xt", [E, B], bf, isOutput=False)
    wt_ext = nc.declare_dram_parameter("wt", [E, CSP], bf, isOutput=False)
    o_acc = nc.declare_dram_parameter("o_acc", [128, NG_TOT], f32, isOutput=True)
    o_max = nc.declare_dram_parameter("o_max", [128, NG_TOT * 8], bf, isOutput=True)

    from contextlib import ExitStack
    with ExitStack() as ctx:
        xt0 = ctx.enter_context(nc.sbuf_tensor("xt0", [128, B], bf))
        xt1 = ctx.enter_context(nc.sbuf_tensor("xt1", [128, B], bf))
        wt0 = ctx.enter_context(nc.sbuf_tensor("wt0", [128, CSP], bf))
        wt1 = ctx.enter_context(nc.sbuf_tensor("wt1", [128, CSP], bf))
        accv = ctx.enter_context(nc.sbuf_tensor("accv", [128, NG_TOT], f32))
        maxb = ctx.enter_context(nc.sbuf_tensor("maxb", [128, NG_TOT * 8], bf))
        tr = ctx.enter_context(nc.sbuf_tensor("tr", [128, 1536], bf))
        expb = [
            ctx.enter_context(nc.sbuf_tensor(f"expb{k}", [128, GFULL], bf))
            for k in range(NEB)
        ]
        ps = [
            ctx.enter_context(nc.psum_tensor(f"ps{k}", [128, GFULL], f32))
            for k in range(2)
        ]
        dma_sem = ctx.enter_context(nc.semaphore("dma_sem"))
        dma2_sem = ctx.enter_context(nc.semaphore("dma2_sem"))
        wt_sems = [ctx.enter_context(nc.semaphore(f"wt_sem{i}")) for i in range(NGRP)]
        mm_sem = ctx.enter_context(nc.semaphore("mm_sem"))
        act_sem = ctx.enter_context(nc.semaphore("act_sem"))
        dve_sem = ctx.enter_context(nc.semaphore("dve_sem"))
        block = ctx.enter_context(nc.Block())

        # first group's W arrives via 4 row-sliced pieces per half (4KB
        # descriptors, parallel queues) for a fast pipeline start
        r0_rows = [slice(j * 32, (j + 1) * 32) for j in range(4)]
        rest = [slice(st, st + ln) for (st, ln) in GROUP_RANGES[1:]]
        Q4 = NG_TOT // 4

        @block.sync
        def _(sync):
            sync.dma_start(out=xt0[:], in_=xt_ext[0:128, :]).then_inc(dma_sem, 16)
            sync.dma_start(out=xt1[:], in_=xt_ext[128:256, :]).then_inc(dma_sem, 16)
            for rs in r0_rows:
                sync.dma_start(out=wt0[rs, 0:GFULL], in_=wt_ext[rs, 0:GFULL]).then_inc(wt_sems[0], 16)
            for gi, sl in enumerate(rest, start=1):
                sync.dma_start(out=wt0[:, sl], in_=wt_ext[0:128, sl]).then_inc(wt_sems[gi], 16)
            sync.wait_ge(act_sem, NG_TOT // 2)
            sync.dma_start(out=o_acc[:, 0:NG_TOT // 2], in_=accv[:, 0:NG_TOT // 2]).then_inc(dma_sem, 16)
            sync.wait_ge(act_sem, NG_TOT)
            sync.dma_start(out=o_acc[:, NG_TOT // 2:], in_=accv[:, NG_TOT // 2:]).then_inc(dma_sem, 16)
            sync.wait_ge(dve_sem, 3 * Q4)
            sync.dma_start(out=o_max[:, 16 * Q4:24 * Q4], in_=maxb[:, 16 * Q4:24 * Q4]).then_inc(dma_sem, 16)
            sync.wait_ge(dve_sem, NG_TOT)
            sync.dma_start(out=o_max[:, 24 * Q4:], in_=maxb[:, 24 * Q4:]).then_inc(dma_sem, 16)

        @block.gpsimd
        def _(gpsimd):
            rs2 = [slice(128 + j * 32, 128 + (j + 1) * 32) for j in range(4)]
            for j, rs in enumerate(rs2):
                gpsimd.dma_start(out=wt1[r0_rows[j], 0:GFULL], in_=wt_ext[rs, 0:GFULL]).then_inc(wt_sems[0], 16)
            for gi, sl in enumerate(rest, start=1):
                gpsimd.dma_start(out=wt1[:, sl], in_=wt_ext[128:256, sl]).then_inc(wt_sems[gi], 16)
            gpsimd.wait_ge(dve_sem, Q4)
            gpsimd.dma_start(out=o_max[:, 0:8 * Q4], in_=maxb[:, 0:8 * Q4]).then_inc(dma2_sem, 16)
            gpsimd.wait_ge(dve_sem, 2 * Q4)
            gpsimd.dma_start(out=o_max[:, 8 * Q4:16 * Q4], in_=maxb[:, 8 * Q4:16 * Q4]).then_inc(dma2_sem, 16)

        @block.tensor
        def _(tensor):
            g = 0
            for t in range(NTILES):
                tsl = slice(t * 128, (t + 1) * 128)
                for gi, (st, ln) in enumerate(GROUP_RANGES):
                    if t == 0:
                        if gi == 0:
                            tensor.wait_ge(dma_sem, 32)          # xt halves
                            tensor.wait_ge(wt_sems[0], 16 * 8)   # 4+4 row pieces
                        else:
                            tensor.wait_ge(wt_sems[gi], 32)      # both halves
                    if g >= 2:
                        tensor.wait_ge(act_sem, g - 1)
                    p = ps[g % 2]
                    widths = [(o, NCCH) for o in range(0, ln, NCCH)]
                    for k, (xt, wt) in enumerate(((xt0, wt0), (xt1, wt1))):
                        for (off, w) in widths:
                            csl = slice(st + off, st + off + w)
                            mm = tensor.matmul(
                                p[:, off:off + w],
                                xt[:, tsl], wt[:, csl],
                                start=(k == 0), stop=(k == 1),
                            )
                    mm.then_inc(mm_sem, 1)
                    g += 1

        @block.scalar
        def _(scalar):
            import concourse.mybir as mybir
            g = 0
            for t in range(NTILES):
                for gi, (st, ln) in enumerate(GROUP_RANGES):
                    scalar.wait_ge(mm_sem, g + 1)
                    if g >= NEB and g % 2 == 0:
                        scalar.wait_ge(dve_sem, g - 4)
                    scalar.activation(
                        expb[g % NEB][:, 0:ln], ps[g % 2][:, 0:ln],
                        mybir.ActivationFunctionType.Exp,
                        scale=SCALE, accum_out=accv[:, g:g + 1],
                    ).then_inc(act_sem, 1)
                    g += 1

        @block.vector
        def _(vector):
            import concourse.mybir as mybir
            mx = mybir.AluOpType.max
            g = 0
            for t in range(NTILES):
                for gi, (st, ln) in enumerate(GROUP_RANGES):
                    vector.wait_ge(act_sem, g + 1)
                    e = expb[g % NEB]
                    if ln == GFULL:
                        vector.tensor_tensor(tr[:, 0:1024], e[:, 0:1024], e[:, 1024:2048], mx)
                        vector.tensor_tensor(tr[:, 1024:1536], tr[:, 0:512], tr[:, 512:1024], mx)
                        vector.tensor_tensor(tr[:, 0:256], tr[:, 1024:1280], tr[:, 1280:1536], mx)
                    else:
                        vector.tensor_tensor(tr[:, 0:256], e[:, 0:256], e[:, 256:512], mx)
                    vector.max(maxb[:, g * 8:(g + 1) * 8], tr[:, 0:256]).then_inc(dve_sem, 1)
                    g += 1

    return nc


def _worker(tmpdir):
    trace = os.environ.get("AML_TRACE", "0") == "1"
    if trace:
        # Dev-only: wire up the NTFF profile hook that this image's antenv
        # lacks, and stub the artifact upload (no bucket access here).
        try:
            import types

            import trn_agent_boot.trn_boot as tb

            hook = tb._ntff_profile_via_ctypes("/opt/axon/libaxon_pjrt.so")
            mod = types.ModuleType("antenv.axon_hooks")
            mod.get_axon_ntff_profile_hook = lambda: hook
            sys.modules["antenv.axon_hooks"] = mod
            import concourse.bass_utils as _bu

            _bu.upload_artifacts = lambda d: "local://" + d
        except Exception as e:  # pragma: no cover
            print("trace setup failed, disabling trace:", e)
            trace = False

    from concourse.bass_utils import run_bass_kernel_spmd

    import ml_dtypes

    xt = np.load(os.path.join(tmpdir, "xt.npy")).view(ml_dtypes.bfloat16)
    wt = np.load(os.path.join(tmpdir, "wt.npy")).view(ml_dtypes.bfloat16)  # [8, E, CSP]
    nc = _build_graph()
    in_maps = [{"xt": xt, "wt": wt[k]} for k in range(NCORES)]
    try:
        res = run_bass_kernel_spmd(nc, in_maps, core_ids=list(range(NCORES)), trace=trace)
    except Exception:
        if not trace:
            raise
        print("trace run failed; retrying without trace")
        res = run_bass_kernel_spmd(nc, in_maps, core_ids=list(range(NCORES)), trace=False)
    acc = np.stack([r["o_acc"] for r in res.results])                      # [8,128,56] f32
    mx = np.stack([r["o_max"].astype(np.float32) for r in res.results])   # [8,128,448]
    np.save(os.path.join(tmpdir, "acc.npy"), acc)
    np.save(os.path.join(tmpdir, "max.npy"), mx)
    if res.exec_time_ns is not None:
        with open(os.path.join(tmpdir, "exec_time_ns.txt"), "w") as f:
            f.write(str(res.exec_time_ns))


def kernel(inputs, targets, W):
    import ml_dtypes

    x = np.asarray(inputs)
    tg = np.asarray(targets).astype(np.int64)
    Wf = np.asarray(W)

    x64 = x.astype(np.float64)
    W64 = Wf.astype(np.float64)
    xn64 = x64 / np.linalg.norm(x64, axis=1, keepdims=True)
    Wn64 = W64 / np.linalg.norm(W64, axis=1, keepdims=True)
    xt = np.ascontiguousarray(xn64.T.astype(ml_dtypes.bfloat16))          # [E, B]
    WnT = Wn64.T.astype(ml_dtypes.bfloat16)                               # [E, C]
    wt = np.zeros((NCORES, E, CSP), dtype=ml_dtypes.bfloat16)
    for k in range(NCORES):
        wt[k, :, :CS] = WnT[:, k * CS:(k + 1) * CS]

    tmpdir = tempfile.mkdtemp(prefix="aml_")
    np.save(os.path.join(tmpdir, "xt.npy"), xt.view(np.uint16))
    np.save(os.path.join(tmpdir, "wt.npy"), wt.view(np.uint16))
    env = dict(os.environ)
    env["JAX_PLATFORMS"] = "axon"
    subprocess.run(
        [sys.executable, os.path.abspath(__file__), "--worker", tmpdir],
        check=True, env=env,
    )
    acc = np.load(os.path.join(tmpdir, "acc.npy"))   # [8,128,56] col = t*7+gi
    mx = np.load(os.path.join(tmpdir, "max.npy"))    # [8,128,448]

    # --- partial sums -> full sum_exp per sample (sample i = t*128 + p) ---
    acc4 = acc.reshape(NCORES, 128, NTILES, NGRP).astype(np.float64)
    acc4[:, :, :, 6] -= float(CSP - CS)              # remove exp(0)=1 pads (rump group)
    sums_cpt = acc4.sum(axis=3)                      # [core, p, t]
    S = sums_cpt.sum(axis=0).T.reshape(B)            # [p,t] -> [t,p] -> flat i=t*128+p

    # group maxima: max of the 8 folded values per group
    mx5 = mx.reshape(NCORES, 128, NTILES, NGRP, 8).max(axis=4)  # [core,p,t,g]
    gmaxv = np.transpose(mx5, (2, 1, 0, 3)).reshape(B, NCORES * NGRP)  # [i, core*7+g]

    # --- argmax via candidate groups + exact rescore ---
    gm = gmaxv.max(axis=1)
    thresh = gm * np.exp(-SCALE * DELTA_COS)
    cand_mask = gmaxv >= thresh[:, None]
    preds = np.full(B, -1, dtype=np.int64)
    best = np.full(B, -np.inf)
    for gg in np.nonzero(cand_mask.any(axis=0))[0]:
        rows = np.nonzero(cand_mask[:, gg])[0]
        core, gi = divmod(gg, NGRP)
        st, ln = GROUP_RANGES[gi]
        base = core * CS + st
        ln = min(ln, CS - st)                        # clip pads off the rump
        blockW = Wn64[base:base + ln]
        scores = xn64[rows] @ blockW.T
        loc = np.argmax(scores, axis=1)
        val = scores[np.arange(len(rows)), loc]
        upd = val > best[rows]
        ridx = rows[upd]
        best[ridx] = val[upd]
        preds[ridx] = base + loc[upd]

    # --- loss ---
    cos_t = np.einsum("ij,ij->i", xn64, Wn64[tg])
    cos_t = np.clip(cos_t, -1.0, 1.0)
    num = SCALE * (np.cos(np.arccos(cos_t) + MARGIN))
    sum_excl = S - np.exp(SCALE * cos_t)
    den = np.exp(num) + sum_excl
    loss = -np.mean(num - np.log(den + EPS))

    return inputs, preds.astype(np.int32), np.float32(loss)


if __name__ == "__main__":
    if len(sys.argv) >= 3 and sys.argv[1] == "--worker":
        _worker(sys.argv[2])


# revision 29
# speedup vs baseline: 1.0462x; 1.0462x over previous
"""AngularMarginLoss (ArcFace-style) distributed Trainium2 kernel.

Strategy (class-parallel over 8 NeuronCores):
  - Host: L2-normalize x rows and W rows (f64), cast to bf16, transpose to
    [E, *] layouts, shard W columns (classes) across 8 cores, zero-pad each
    shard 12500 -> 12800 columns (25 x 512, PSUM-bank aligned; pad cols give
    exp(0)=1.0 exactly, subtracted on host).
  - Device (per core):
      bf16 matmul (xnT tile stationary, WnT chunk moving) -> PSUM f32
      [128, 2048] groups (4 banks, double buffered)
      ScalarE: exp(30*cos) PSUM->SBUF bf16 + accum_out -> per-group sums
      VectorE: tensor_tensor max tree (bf16 2x mode) + MAX8 -> per-group max
  - Host epilogue: combine partial sums (pad-corrected); resolve argmax by
    rescoring candidate groups (within exp(-30*3e-3) of max) in f64 — this
    absorbs all bf16/exp rounding; compute margin numerator and loss.

Self-contained: shapes hardcoded; device work runs in a subprocess so the
harness's jax state/env cannot interfere.
"""
import os
import subprocess
import sys
import tempfile

import numpy as np

B, E, C = 1024, 256, 100000
NCORES = 8
CS = C // NCORES            # 12500 real classes per core
CSP = 12544                 # padded: 512 + 5*2048 + 1792 (44 pad cols)
NCCH = 512                  # classes per matmul (ISA cap: 512 out elements)
GFULL = 2048                # full group = 4 matmuls = 4 banks
NGRP = 7                    # per tile: 512 first (fast start), 5 x 2048, 1792
GROUP_RANGES = [(0, 512)] + [(512 + g * GFULL, GFULL) for g in range(5)] + [(10752, 1792)]
NTILES = B // 128           # 8 B-tiles
NG_TOT = NTILES * NGRP      # 56
NEB = 6                     # exp buffers
SCALE = 30.0
MARGIN = 0.5
EPS = 1e-6
DELTA_COS = 3e-3            # bf16 error margin for argmax group candidates


def _build_graph():
    import concourse.bass as bass
    import concourse.mybir as mybir

    nc = bass.Bass(target_bir_lowering=False)
    bf = mybir.dt.bfloat16
    f32 = mybir.dt.float32
    xt_ext = nc.declare_dram_parameter("xt", [E, B], bf, isOutput=False)
    wt_ext = nc.declare_dram_parameter("wt", [E, CSP], bf, isOutput=False)
    o_acc = nc.declare_dram_parameter("o_acc", [128, NG_TOT], f32, isOutput=True)
    o_max = nc.declare_dram_parameter("o_max", [128, NG_TOT * 8], bf, isOutput=True)

    from contextlib import ExitStack
    with ExitStack() as ctx:
        xt0 = ctx.enter_context(nc.sbuf_tensor("xt0", [128, B], bf))
        xt1 = ctx.enter_context(nc.sbuf_tensor("xt1", [128, B], bf))
        wt0 = ctx.enter_context(nc.sbuf_tensor("wt0", [128, CSP], bf))
        wt1 = ctx.enter_context(nc.sbuf_tensor("wt1", [128, CSP], bf))
        accv = ctx.enter_context(nc.sbuf_tensor("accv", [128, NG_TOT], f32))
        maxb = ctx.enter_context(nc.sbuf_tensor("maxb", [128, NG_TOT * 8], bf))
        tr = ctx.enter_context(nc.sbuf_tensor("tr", [128, 1536], bf))
        expb = [
            ctx.enter_context(nc.sbuf_tensor(f"expb{k}", [128, GFULL], bf))
            for k in range(NEB)
        ]
        ps = [
            ctx.enter_context(nc.psum_tensor(f"ps{k}", [128, GFULL], f32))
            for k in range(2)
        ]
        dma_sem = ctx.enter_context(nc.semaphore("dma_sem"))
        dma2_sem = ctx.enter_context(nc.semaphore("dma2_sem"))
        wt_sems = [ctx.enter_context(nc.semaphore(f"wt_sem{i}")) for i in range(NGRP)]
        mm_sem = ctx.enter_context(nc.semaphore("mm_sem"))
        act_sem = ctx.enter_context(nc.semaphore("act_sem"))
        dve_sem = ctx.enter_context(nc.semaphore("dve_sem"))
        block = ctx.enter_context(nc.Block())

        # W ranges 0 and 1 arrive as 2 row-sliced pieces per half (parallel
        # queues, big descriptors) so the pipeline starts early; rest whole.
        rh = [slice(0, 64), slice(64, 128)]
        rest = [slice(st, st + ln) for (st, ln) in GROUP_RANGES[2:]]
        Q4 = NG_TOT // 4

        @block.sync
        def _(sync):
            sync.dma_start(out=xt0[:], in_=xt_ext[0:128, :]).then_inc(dma_sem, 16)
            sync.dma_start(out=xt1[:], in_=xt_ext[128:256, :]).then_inc(dma_sem, 16)
            for rs in rh:
                sync.dma_start(out=wt0[rs, 0:512], in_=wt_ext[rs, 0:512]).then_inc(wt_sems[0], 16)
            for rs in rh:
                sync.dma_start(out=wt0[rs, 512:2560], in_=wt_ext[rs, 512:2560]).then_inc(wt_sems[1], 16)
            for gi, sl in enumerate(rest, start=2):
                sync.dma_start(out=wt0[:, sl], in_=wt_ext[0:128, sl]).then_inc(wt_sems[gi], 16)
            sync.wait_ge(act_sem, NG_TOT // 2)
            sync.dma_start(out=o_acc[:, 0:NG_TOT // 2], in_=accv[:, 0:NG_TOT // 2]).then_inc(dma_sem, 16)
            sync.wait_ge(act_sem, NG_TOT)
            sync.dma_start(out=o_acc[:, NG_TOT // 2:], in_=accv[:, NG_TOT // 2:]).then_inc(dma_sem, 16)
            sync.wait_ge(dve_sem, 3 * Q4)
            sync.dma_start(out=o_max[:, 16 * Q4:24 * Q4], in_=maxb[:, 16 * Q4:24 * Q4]).then_inc(dma_sem, 16)
            sync.wait_ge(dve_sem, NG_TOT)
            sync.dma_start(out=o_max[:, 24 * Q4:], in_=maxb[:, 24 * Q4:]).then_inc(dma_sem, 16)

        @block.gpsimd
        def _(gpsimd):
            for rs in rh:
                src = slice(128 + rs.start, 128 + rs.stop)
                gpsimd.dma_start(out=wt1[rs, 0:512], in_=wt_ext[src, 0:512]).then_inc(wt_sems[0], 16)
            for rs in rh:
                src = slice(128 + rs.start, 128 + rs.stop)
                gpsimd.dma_start(out=wt1[rs, 512:2560], in_=wt_ext[src, 512:2560]).then_inc(wt_sems[1], 16)
            for gi, sl in enumerate(rest, start=2):
                gpsimd.dma_start(out=wt1[:, sl], in_=wt_ext[128:256, sl]).then_inc(wt_sems[gi], 16)
            gpsimd.wait_ge(dve_sem, Q4)
            gpsimd.dma_start(out=o_max[:, 0:8 * Q4], in_=maxb[:, 0:8 * Q4]).then_inc(dma2_sem, 16)
            gpsimd.wait_ge(dve_sem, 2 * Q4)
            gpsimd.dma_start(out=o_max[:, 8 * Q4:16 * Q4], in_=maxb[:, 8 * Q4:16 * Q4]).then_inc(dma2_sem, 16)

        @block.tensor
        def _(tensor):
            g = 0
            for t in range(NTILES):
                tsl = slice(t * 128, (t + 1) * 128)
                for gi, (st, ln) in enumerate(GROUP_RANGES):
                    if t == 0:
                        if gi == 0:
                            tensor.wait_ge(dma_sem, 32)          # xt halves
                            tensor.wait_ge(wt_sems[0], 16 * 4)   # 2+2 row pieces
                        elif gi == 1:
                            tensor.wait_ge(wt_sems[1], 16 * 4)   # 2+2 row pieces
                        else:
                            tensor.wait_ge(wt_sems[gi], 32)      # both halves
                    if g >= 2:
                        tensor.wait_ge(act_sem, g - 1)
                    p = ps[g % 2]
                    widths = [(o, min(NCCH, ln - o)) for o in range(0, ln, NCCH)]
                    for k, (xt, wt) in enumerate(((xt0, wt0), (xt1, wt1))):
                        for (off, w) in widths:
                            csl = slice(st + off, st + off + w)
                            mm = tensor.matmul(
                                p[:, off:off + w],
                                xt[:, tsl], wt[:, csl],
                                start=(k == 0), stop=(k == 1),
                            )
                    mm.then_inc(mm_sem, 1)
                    g += 1

        @block.scalar
        def _(scalar):
            import concourse.mybir as mybir
            g = 0
            for t in range(NTILES):
                for gi, (st, ln) in enumerate(GROUP_RANGES):
                    scalar.wait_ge(mm_sem, g + 1)
                    if g >= NEB and g % 2 == 0:
                        scalar.wait_ge(dve_sem, g - 4)
                    scalar.activation(
                        expb[g % NEB][:, 0:ln], ps[g % 2][:, 0:ln],
                        mybir.ActivationFunctionType.Exp,
                        scale=SCALE, accum_out=accv[:, g:g + 1],
                    ).then_inc(act_sem, 1)
                    g += 1

        @block.vector
        def _(vector):
            import concourse.mybir as mybir
            mx = mybir.AluOpType.max
            g = 0
            for t in range(NTILES):
                for gi, (st, ln) in enumerate(GROUP_RANGES):
                    vector.wait_ge(act_sem, g + 1)
                    e = expb[g % NEB]
                    if ln == GFULL:
                        vector.tensor_tensor(tr[:, 0:1024], e[:, 0:1024], e[:, 1024:2048], mx)
                        vector.tensor_tensor(tr[:, 1024:1536], tr[:, 0:512], tr[:, 512:1024], mx)
                        vector.tensor_tensor(tr[:, 0:256], tr[:, 1024:1280], tr[:, 1280:1536], mx)
                        fin = tr[:, 0:256]
                    elif ln == 1792:
                        vector.tensor_tensor(tr[:, 0:896], e[:, 0:896], e[:, 896:1792], mx)
                        vector.tensor_tensor(tr[:, 1024:1472], tr[:, 0:448], tr[:, 448:896], mx)
                        vector.tensor_tensor(tr[:, 0:224], tr[:, 1024:1248], tr[:, 1248:1472], mx)
                        fin = tr[:, 0:224]
                    else:
                        vector.tensor_tensor(tr[:, 0:256], e[:, 0:256], e[:, 256:512], mx)
                        fin = tr[:, 0:256]
                    vector.max(maxb[:, g * 8:(g + 1) * 8], fin).then_inc(dve_sem, 1)
                    g += 1

    return nc


def _worker(tmpdir):
    trace = os.environ.get("AML_TRACE", "0") == "1"
    if trace:
        # Dev-only: wire up the NTFF profile hook that this image's antenv
        # lacks, and stub the artifact upload (no bucket access here).
        try:
            import types

            import trn_agent_boot.trn_boot as tb

            hook = tb._ntff_profile_via_ctypes("/opt/axon/libaxon_pjrt.so")
            mod = types.ModuleType("antenv.axon_hooks")
            mod.get_axon_ntff_profile_hook = lambda: hook
            sys.modules["antenv.axon_hooks"] = mod
            import concourse.bass_utils as _bu

            _bu.upload_artifacts = lambda d: "local://" + d
        except Exception as e:  # pragma: no cover
            print("trace setup failed, disabling trace:", e)
            trace = False

    from concourse.bass_utils import run_bass_kernel_spmd

    import ml_dtypes

    xt = np.load(os.path.join(tmpdir, "xt.npy")).view(ml_dtypes.bfloat16)
    wt = np.load(os.path.join(tmpdir, "wt.npy")).view(ml_dtypes.bfloat16)  # [8, E, CSP]
    nc = _build_graph()
    in_maps = [{"xt": xt, "wt": wt[k]} for k in range(NCORES)]
    try:
        res = run_bass_kernel_spmd(nc, in_maps, core_ids=list(range(NCORES)), trace=trace)
    except Exception:
        if not trace:
            raise
        print("trace run failed; retrying without trace")
        res = run_bass_kernel_spmd(nc, in_maps, core_ids=list(range(NCORES)), trace=False)
    acc = np.stack([r["o_acc"] for r in res.results])                      # [8,128,56] f32
    mx = np.stack([r["o_max"].astype(np.float32) for r in res.results])   # [8,128,448]
    np.save(os.path.join(tmpdir, "acc.npy"), acc)
    np.save(os.path.join(tmpdir, "max.npy"), mx)
    if res.exec_time_ns is not None:
        with open(os.path.join(tmpdir, "exec_time_ns.txt"), "w") as f:
            f.write(str(res.exec_time_ns))


def kernel(inputs, targets, W):
    import ml_dtypes

    x = np.asarray(inputs)
    tg = np.asarray(targets).astype(np.int64)
    Wf = np.asarray(W)

    x64 = x.astype(np.float64)
    W64 = Wf.astype(np.float64)
    xn64 = x64 / np.linalg.norm(x64, axis=1, keepdims=True)
    Wn64 = W64 / np.linalg.norm(W64, axis=1, keepdims=True)
    xt = np.ascontiguousarray(xn64.T.astype(ml_dtypes.bfloat16))          # [E, B]
    WnT = Wn64.T.astype(ml_dtypes.bfloat16)                               # [E, C]
    wt = np.zeros((NCORES, E, CSP), dtype=ml_dtypes.bfloat16)
    for k in range(NCORES):
        wt[k, :, :CS] = WnT[:, k * CS:(k + 1) * CS]

    tmpdir = tempfile.mkdtemp(prefix="aml_")
    np.save(os.path.join(tmpdir, "xt.npy"), xt.view(np.uint16))
    np.save(os.path.join(tmpdir, "wt.npy"), wt.view(np.uint16))
    env = dict(os.environ)
    env["JAX_PLATFORMS"] = "axon"
    subprocess.run(
        [sys.executable, os.path.abspath(__file__), "--worker", tmpdir],
        check=True, env=env,
    )
    acc = np.load(os.path.join(tmpdir, "acc.npy"))   # [8,128,56] col = t*7+gi
    mx = np.load(os.path.join(tmpdir, "max.npy"))    # [8,128,448]

    # --- partial sums -> full sum_exp per sample (sample i = t*128 + p) ---
    acc4 = acc.reshape(NCORES, 128, NTILES, NGRP).astype(np.float64)
    acc4[:, :, :, 6] -= float(CSP - CS)              # remove exp(0)=1 pads (rump group)
    sums_cpt = acc4.sum(axis=3)                      # [core, p, t]
    S = sums_cpt.sum(axis=0).T.reshape(B)            # [p,t] -> [t,p] -> flat i=t*128+p

    # group maxima: max of the 8 folded values per group
    mx5 = mx.reshape(NCORES, 128, NTILES, NGRP, 8).max(axis=4)  # [core,p,t,g]
    gmaxv = np.transpose(mx5, (2, 1, 0, 3)).reshape(B, NCORES * NGRP)  # [i, core*7+g]

    # --- argmax via candidate groups + exact rescore ---
    gm = gmaxv.max(axis=1)
    thresh = gm * np.exp(-SCALE * DELTA_COS)
    cand_mask = gmaxv >= thresh[:, None]
    preds = np.full(B, -1, dtype=np.int64)
    best = np.full(B, -np.inf)
    for gg in np.nonzero(cand_mask.any(axis=0))[0]:
        rows = np.nonzero(cand_mask[:, gg])[0]
        core, gi = divmod(gg, NGRP)
        st, ln = GROUP_RANGES[gi]
        base = core * CS + st
        ln = min(ln, CS - st)                        # clip pads off the rump
        blockW = Wn64[base:base + ln]
        scores = xn64[rows] @ blockW.T
        loc = np.argmax(scores, axis=1)
        val = scores[np.arange(len(rows)), loc]
        upd = val > best[rows]
        ridx = rows[upd]
        best[ridx] = val[upd]
        preds[ridx] = base + loc[upd]

    # --- loss ---
    cos_t = np.einsum("ij,ij->i", xn64, Wn64[tg])
    cos_t = np.clip(cos_t, -1.0, 1.0)
    num = SCALE * (np.cos(np.arccos(cos_t) + MARGIN))
    sum_excl = S - np.exp(SCALE * cos_t)
    den = np.exp(num) + sum_excl
    loss = -np.mean(num - np.log(den + EPS))

    return inputs, preds.astype(np.int32), np.float32(loss)


if __name__ == "__main__":
    if len(sys.argv) >= 3 and sys.argv[1] == "--worker":
        _worker(sys.argv[2])
